# revision 1
# baseline (speedup 1.0000x reference)
"""Trainium2 Bass kernel for nn_CDE: natural-cubic-spline CDE with RK4(3/8) solver.

Strategy:
- Host: fold the spline solve into a fixed (60, 16) matrix C60 over the time
  axis (derived from t alone): every RK-stage derivative dX/dt is a linear
  combination of the 16 knots of x.  Pack only mask-active trajectories,
  pad to 8*Nc (Nc in {128, 256}), shard across 8 cores.
- Device (per core): feature-major MLP (layers 0-2: [feat_part, batch_free]),
  batch-major layer 3 (stationary = activations) with W3 rows permuted
  d-major so the einsum over D becomes 10 fused DVE scalar_tensor_tensor ops
  with per-partition dX scalars.  PE transpose brings k back to
  feature-major; RK4 combinations are fused scalar_tensor_tensor ops.
- Matmuls run in float32r (TF32-class, ~1.5e-4 rel err, full-rate on PE) or
  bf16 (env KERNEL_DT=bf16).
"""
import os
import sys
import types

for _p in ("/opt/trn_rl_repo", "/root/.axon_site/_ro/trn_rl_repo"):
    if os.path.isdir(_p) and _p not in sys.path:
        sys.path.insert(0, _p)

# antenv.axon_hooks shim so BASS_TRACE=1 works under axon (missing in image)
if "antenv.axon_hooks" not in sys.modules:
    _m = types.ModuleType("antenv.axon_hooks")
    _hook = [None]

    def _set(hook):
        _hook[0] = hook

    def _get():
        if _hook[0] is None:
            try:
                from trn_agent_boot.trn_boot import _ntff_profile_via_ctypes
                _hook[0] = _ntff_profile_via_ctypes("/opt/axon/libaxon_pjrt.so")
            except Exception:
                pass
        return _hook[0]

    _m.set_axon_ntff_profile_hook = _set
    _m.get_axon_ntff_profile_hook = _get
    sys.modules["antenv.axon_hooks"] = _m

import numpy as np

N_CORES = 8
T, D, E, H = 16, 10, 128, 512
F3 = E * D  # 1280
N_STEPS = T - 1
N_STAGES = 4 * N_STEPS  # 60
SLICES = [(0, 512), (512, 1024), (1024, 1280)]

last_results = None


def spline_stage_matrix(t):
    """C60 (60,16): row 4j+r maps the 16 knots of one scalar series to the
    spline derivative at RK stage r of step j.  Also returns h (15,)."""
    t = np.asarray(t, np.float64)
    Tn = len(t)
    h = np.diff(t)
    A = np.zeros((Tn, Tn))
    A[0, 0] = 1.0
    A[-1, -1] = 1.0
    for i in range(1, Tn - 1):
        A[i, i - 1] = h[i - 1]
        A[i, i] = 2.0 * (h[i - 1] + h[i])
        A[i, i + 1] = h[i]
    R = np.zeros((Tn, Tn))
    for i in range(1, Tn - 1):
        R[i, i - 1] = 6.0 / h[i - 1]
        R[i, i] = -6.0 / h[i - 1] - 6.0 / h[i]
        R[i, i + 1] = 6.0 / h[i]
    S = np.linalg.solve(A, R)  # M = S @ x  (second derivatives)
    Iden = np.eye(Tn)
    rows = []
    for j in range(Tn - 1):
        hs = h[j]
        for u_frac in (0.0, 1.0 / 3.0, 2.0 / 3.0, 1.0):
            s = t[j + 1] if u_frac == 1.0 else t[j] + u_frac * hs
            i = int(np.clip(np.searchsorted(t, s, side="right") - 1, 0, Tn - 2))
            u = s - t[i]
            b_row = (Iden[i + 1] - Iden[i]) / h[i] - h[i] * (2.0 * S[i] + S[i + 1]) / 6.0
            rows.append(b_row + u * S[i] + (u * u) / (2.0 * h[i]) * (S[i + 1] - S[i]))
    return np.asarray(rows), h


def w3_perm():
    """Permutation so W3p[f'] = W3[e*10+d] with f' = d*128+e (d-major)."""
    fp = np.arange(F3)
    return (fp % E) * D + fp // E


def rk4_weights_sim(x_pack, C60, h, W_embed, b_embed, W0, b0, W1, b1, W2, b2, W3, b3):
    """Numpy simulation of the exact device math (fp32-ish, fp64 accum)."""
    n = x_pack.shape[0]
    dx_all = np.einsum("st,ntd->snd", C60, x_pack)  # (60, n, 10)
    z = x_pack[:, 0, :] @ W_embed.T + b_embed

    def f(zz):
        y = np.maximum(zz @ W0.T + b0, 0)
        y = np.maximum(y @ W1.T + b1, 0)
        y = np.maximum(y @ W2.T + b2, 0)
        y = np.tanh(y @ W3.T + b3)
        return y.reshape(n, E, D)

    for j in range(N_STEPS):
        hs = h[j]
        k1 = np.einsum("ned,nd->ne", f(z), dx_all[4 * j + 0])
        k2 = np.einsum("ned,nd->ne", f(z + hs * k1 / 3.0), dx_all[4 * j + 1])
        k3 = np.einsum("ned,nd->ne", f(z + hs * (k2 - k1 / 3.0)), dx_all[4 * j + 2])
        k4 = np.einsum("ned,nd->ne", f(z + hs * (k1 - k2 + k3)), dx_all[4 * j + 3])
        z = z + hs * (k1 + 3.0 * (k2 + k3) + k4) / 8.0
    return z


def build_bass(Nc, dt_name, h, dve_writes_dt=True):
    """Build the per-core SPMD Bass program (fully unrolled 60 stages)."""
    import concourse.bacc as bacc
    import concourse.tile as tile
    import concourse.mybir as mybir
    from concourse.masks import make_identity

    F32 = mybir.dt.float32
    F32R = mybir.dt.float32r
    BF16 = mybir.dt.bfloat16
    AF = mybir.ActivationFunctionType
    ALU = mybir.AluOpType
    DT = {"f32r": F32R, "bf16": BF16}[dt_name]

    NT = Nc // 128
    nc = bacc.Bacc("TRN2", target_bir_lowering=False)

    d_xbyd = nc.dram_tensor("x_byd", [T, D, Nc], F32, kind="ExternalInput")
    d_xt0 = nc.dram_tensor("x_t0", [D, Nc], F32, kind="ExternalInput")
    d_c60 = nc.dram_tensor("c60t", [T, N_STAGES], F32, kind="ExternalInput")
    d_wemb = nc.dram_tensor("w_embt", [D, E], F32, kind="ExternalInput")
    d_bemb = nc.dram_tensor("b_emb", [E, 1], F32, kind="ExternalInput")
    d_w0 = nc.dram_tensor("w0t", [E, H], DT, kind="ExternalInput")
    d_w1 = nc.dram_tensor("w1t", [H, H], DT, kind="ExternalInput")
    d_w2 = nc.dram_tensor("w2t", [H, H], DT, kind="ExternalInput")
    d_w3 = nc.dram_tensor("w3pt", [H, F3], DT, kind="ExternalInput")
    d_b012 = nc.dram_tensor("b012", [E, 12], F32, kind="ExternalInput")
    d_ones4 = nc.dram_tensor("ones4", [4, E], F32R, kind="ExternalInput")
    d_b3p4 = nc.dram_tensor("b3p4", [4, F3], F32R, kind="ExternalInput")
    d_out = nc.dram_tensor("zout", [E, Nc], F32, kind="ExternalOutput")

    with tile.TileContext(nc) as tc:
        with (
            tc.tile_pool(name="wpool", bufs=1) as wpool,
            tc.tile_pool(name="xpool", bufs=1) as xpool,
            tc.tile_pool(name="apool", bufs=2) as apool,
            tc.tile_pool(name="pmlp", bufs=(3 if NT == 1 else 2), space="PSUM") as pmlp,
            tc.tile_pool(name="p3p", bufs=3, space="PSUM") as p3p,
            tc.tile_pool(name="ptrp", bufs=(2 if NT == 1 else 1), space="PSUM") as ptrp,
        ):
            # ---- load constants / weights
            w0t = wpool.tile([E, H], DT, tag="w0t")
            nc.sync.dma_start(out=w0t, in_=d_w0[:, :])
            w1k = [wpool.tile([128, H], DT, tag=f"w1k{k}", name=f"w1k{k}")
                   for k in range(4)]
            w2k = [wpool.tile([128, H], DT, tag=f"w2k{k}", name=f"w2k{k}")
                   for k in range(4)]
            w3k = [wpool.tile([128, F3], DT, tag=f"w3k{k}", name=f"w3k{k}")
                   for k in range(4)]
            for k in range(4):
                nc.sync.dma_start(out=w1k[k], in_=d_w1[128 * k:128 * (k + 1), :])
                nc.sync.dma_start(out=w2k[k], in_=d_w2[128 * k:128 * (k + 1), :])
                nc.sync.dma_start(out=w3k[k], in_=d_w3[128 * k:128 * (k + 1), :])
            b012 = wpool.tile([E, 12], F32, tag="b012")
            nc.sync.dma_start(out=b012, in_=d_b012[:, :])
            bemb = wpool.tile([E, 1], F32, tag="bemb")
            nc.sync.dma_start(out=bemb, in_=d_bemb[:, :])
            ones4 = wpool.tile([4, E], F32R, tag="ones4")
            nc.sync.dma_start(out=ones4, in_=d_ones4[:, :])
            b3p4 = wpool.tile([4, F3], F32R, tag="b3p4")
            nc.sync.dma_start(out=b3p4, in_=d_b3p4[:, :])
            wembt = wpool.tile([D, E], F32, tag="wembt")
            nc.sync.dma_start(out=wembt, in_=d_wemb[:, :])
            xbyd = xpool.tile([T, D, Nc], F32, tag="xbyd")
            nc.sync.dma_start(out=xbyd, in_=d_xbyd[:, :, :])
            xt0 = xpool.tile([D, Nc], F32, tag="xt0")
            nc.sync.dma_start(out=xt0, in_=d_xt0[:, :])
            c60 = xpool.tile([T, N_STAGES], F32, tag="c60")
            nc.sync.dma_start(out=c60, in_=d_c60[:, :])
            ident = wpool.tile([128, 128], F32, tag="ident")
            make_identity(nc, ident)

            # ---- spline: DXb[nt][:, d, s] = dX/dt for traj (nt,part), coord d, stage s
            DXb = [xpool.tile([128, D, N_STAGES], F32, tag=f"dxb{nt}", name=f"dxb{nt}")
                   for nt in range(NT)]
            for nt in range(NT):
                for d in range(D):
                    pdx = ptrp.tile([128, 256], F32, tag="ptr", name=f"pdx{nt}_{d}")
                    nc.tensor.matmul(pdx[:, 0:N_STAGES],
                                     xbyd[:, d, 128 * nt:128 * (nt + 1)],
                                     c60[:, :], start=True, stop=True)
                    nc.vector.tensor_copy(DXb[nt][:, d, :], pdx[:, 0:N_STAGES])

            # ---- embed: z0 = W_embed @ x(t0) + b  (feature-major [E, Nc])
            pemb = ptrp.tile([128, 256], F32, tag="ptr", name="pemb")
            nc.tensor.matmul(pemb[:, 0:Nc], wembt[:, :], xt0[:, :],
                             start=True, stop=True)
            z = apool.tile([E, Nc], DT, tag="z", name="z0")
            nc.scalar.activation(z, pemb[:, 0:Nc], AF.Identity,
                                 bias=bemb[:, :], scale=1.0)

            def relu(engine, out_ap, in_ap, bias_ap):
                if engine == "act":
                    nc.scalar.activation(out_ap, in_ap, AF.Relu,
                                         bias=bias_ap, scale=1.0)
                else:
                    nc.vector.tensor_scalar(out=out_ap, in0=in_ap,
                                            scalar1=bias_ap, scalar2=0.0,
                                            op0=ALU.add, op1=ALU.max)

            RELU_ENG = ["act", "dve", "act"] if dve_writes_dt else ["act", "act", "act"]
            STT_DT = dve_writes_dt  # DVE scalar_tensor_tensor may write DT

            def stt(out_ap, in0_ap, scalar, in1_ap):
                nc.vector.scalar_tensor_tensor(
                    out=out_ap, in0=in0_ap, scalar=scalar, in1=in1_ap,
                    op0=mybir.AluOpType.mult, op1=mybir.AluOpType.add)

            def make_zin(name, s, k_ap, coef, src_z):
                """z-input tile (DT) = (k_ap * coef) + src_z"""
                if STT_DT:
                    zt = apool.tile([E, Nc], DT, tag="zs", name=name)
                    stt(zt, k_ap, coef, src_z)
                    return zt
                zf = apool.tile([E, Nc], F32, tag="zsf", name=name + "_f")
                stt(zf, k_ap, coef, src_z)
                zt = apool.tile([E, Nc], DT, tag="zs", name=name)
                nc.scalar.activation(zt, zf, AF.Identity, bias=0.0, scale=1.0)
                return zt

            ks = [None, None, None, None]  # k1..k4 (feature-major F32 [E, Nc])
            s1t = None

            for j in range(N_STEPS):
                hs = float(h[j])
                for r in range(4):
                    s = 4 * j + r
                    if r == 0:
                        zin = z
                    elif r == 1:
                        zin = make_zin(f"z2in_{s}", s, ks[0], hs / 3.0, z)
                    elif r == 2:
                        tmpA = apool.tile([E, Nc], F32, tag="tmpA", name=f"tmpA_{s}")
                        stt(tmpA, ks[0], -1.0 / 3.0, ks[1])
                        zin = make_zin(f"z3in_{s}", s, tmpA, hs, z)
                    else:
                        tmpB = apool.tile([E, Nc], F32, tag="tmpB", name=f"tmpB_{s}")
                        stt(tmpB, ks[1], -1.0, ks[0])
                        tmpC = apool.tile([E, Nc], F32, tag="tmpC", name=f"tmpC_{s}")
                        stt(tmpC, ks[2], 1.0, tmpB)
                        zin = make_zin(f"z4in_{s}", s, tmpC, hs, z)

                    # ---- MLP layers 0-2 (feature-major)
                    p0 = pmlp.tile([128, 4, Nc], F32, tag="pmlp", name=f"p0_{s}")
                    for m in range(4):
                        nc.tensor.matmul(p0[:, m, :], w0t[:, 128 * m:128 * (m + 1)],
                                         zin[:, :], start=True, stop=True)
                    y0 = apool.tile([128, 4, Nc], DT, tag="y0", name=f"y0_{s}")
                    for m in range(4):
                        relu(RELU_ENG[0], y0[:, m, :], p0[:, m, :],
                             b012[:, 0 + m:1 + m])
                    p1 = pmlp.tile([128, 4, Nc], F32, tag="pmlp", name=f"p1_{s}")
                    for m in range(4):
                        for k in range(4):
                            nc.tensor.matmul(p1[:, m, :],
                                             w1k[k][:, 128 * m:128 * (m + 1)],
                                             y0[:, k, :],
                                             start=(k == 0), stop=(k == 3))
                    y1 = apool.tile([128, 4, Nc], DT, tag="y1", name=f"y1_{s}")
                    for m in range(4):
                        relu(RELU_ENG[1], y1[:, m, :], p1[:, m, :],
                             b012[:, 4 + m:5 + m])
                    p2 = pmlp.tile([128, 4, Nc], F32, tag="pmlp", name=f"p2_{s}")
                    for m in range(4):
                        for k in range(4):
                            nc.tensor.matmul(p2[:, m, :],
                                             w2k[k][:, 128 * m:128 * (m + 1)],
                                             y1[:, k, :],
                                             start=(k == 0), stop=(k == 3))
                    y2 = apool.tile([128, 4, Nc], DT, tag="y2", name=f"y2_{s}")
                    for m in range(4):
                        relu(RELU_ENG[2], y2[:, m, :], p2[:, m, :],
                             b012[:, 8 + m:9 + m])

                    # ---- layer 3 batch-major + einsum + transpose per n-chunk
                    k_fm = apool.tile([E, Nc], F32, tag=f"kst{r}", name=f"k_{s}")
                    for nt in range(NT):
                        y3t = apool.tile([128, F3], F32, tag=f"y3t{nt}",
                                         name=f"y3t_{s}_{nt}")
                        for (sl0, sl1) in SLICES:
                            w = sl1 - sl0
                            p3 = p3p.tile([128, 512], F32, tag="p3",
                                          name=f"p3_{s}_{nt}_{sl0}")
                            for k in range(4):
                                nc.tensor.matmul(p3[:, 0:w],
                                                 y2[:, k, 128 * nt:128 * (nt + 1)],
                                                 w3k[k][:, sl0:sl1],
                                                 start=(k == 0), stop=False)
                            nc.tensor.matmul(p3[:, 0:w], ones4[:, :],
                                             b3p4[:, sl0:sl1],
                                             start=False, stop=True)
                            nc.scalar.activation(y3t[:, sl0:sl1], p3[:, 0:w],
                                                 AF.Tanh)
                        acc = apool.tile([128, 128], F32, tag=f"acc{nt}",
                                         name=f"acc_{s}_{nt}")
                        nc.vector.tensor_scalar(
                            out=acc, in0=y3t[:, 0:128],
                            scalar1=DXb[nt][:, 0, s:s + 1], scalar2=None,
                            op0=mybir.AluOpType.mult)
                        for d in range(1, D):
                            stt(acc, y3t[:, 128 * d:128 * (d + 1)],
                                DXb[nt][:, d, s:s + 1], acc)
                        ktr = ptrp.tile([128, 256], F32, tag="ptr",
                                        name=f"ktr_{s}_{nt}")
                        nc.tensor.transpose(ktr[:, 0:128], acc, ident)
                        nc.vector.tensor_copy(k_fm[:, 128 * nt:128 * (nt + 1)],
                                              ktr[:, 0:128])
                    ks[r] = k_fm

                    if r == 2:
                        s1t = apool.tile([E, Nc], F32, tag="s1t", name=f"s1t_{s}")
                        stt(s1t, ks[1], 1.0, ks[2])  # k2 + k3
                    if r == 3:
                        s2t = apool.tile([E, Nc], F32, tag="s2t", name=f"s2t_{s}")
                        stt(s2t, s1t, 3.0, ks[0])    # 3(k2+k3) + k1
                        s3t = apool.tile([E, Nc], F32, tag="s3t", name=f"s3t_{s}")
                        stt(s3t, ks[3], 1.0, s2t)    # + k4
                        last = (j == N_STEPS - 1)
                        if last:
                            znew = apool.tile([E, Nc], F32, tag="zfin", name="zfin")
                            stt(znew, s3t, hs / 8.0, z)
                        elif STT_DT:
                            znew = apool.tile([E, Nc], DT, tag="z", name=f"z_{s}")
                            stt(znew, s3t, hs / 8.0, z)
                        else:
                            znew = make_zin(f"z_{s}", s, s3t, hs / 8.0, z)
                        z = znew

            nc.sync.dma_start(out=d_out[:, :], in_=z)
    nc.finalize()
    return nc


def _prep_host(t, x, mask, W_embed, b_embed, W0, b0, W1, b1, W2, b2, W3, b3,
               dt_name):
    import ml_dtypes
    wdt = {"f32r": np.float32, "bf16": ml_dtypes.bfloat16}[dt_name]

    t = np.asarray(t, np.float32)
    x = np.asarray(x, np.float32)
    mask = np.asarray(mask)
    B, Amax = mask.shape
    N = B * Amax

    C60, h = spline_stage_matrix(t)
    idx = np.flatnonzero(mask.ravel())
    nact = len(idx)
    Nc = 128 if nact <= N_CORES * 128 else 256
    total = N_CORES * Nc
    pad = np.full(total, idx[0] if nact else 0, dtype=np.int64)
    pad[:nact] = idx
    xp = x.reshape(N, T, D)[pad]  # (total, 16, 10)

    perm = w3_perm()
    shared = dict(
        c60t=np.ascontiguousarray(C60.T.astype(np.float32)),
        w_embt=np.ascontiguousarray(W_embed.T.astype(np.float32)),
        b_emb=np.asarray(b_embed, np.float32).reshape(E, 1),
        w0t=np.ascontiguousarray(np.asarray(W0).T).astype(wdt),
        w1t=np.ascontiguousarray(np.asarray(W1).T).astype(wdt),
        w2t=np.ascontiguousarray(np.asarray(W2).T).astype(wdt),
        w3pt=np.ascontiguousarray(np.asarray(W3)[perm].T).astype(wdt),
        b012=np.stack([np.asarray(b)[m * 128:(m + 1) * 128]
                       for b in (b0, b1, b2) for m in range(4)],
                      axis=1).astype(np.float32),
        ones4=np.full((4, E), 0.25, np.float32),
        b3p4=np.tile(np.asarray(b3, np.float32)[perm][None, :], (4, 1)),
    )
    in_maps = []
    for c in range(N_CORES):
        xc = xp[c * Nc:(c + 1) * Nc]  # (Nc, 16, 10)
        in_maps.append(dict(
            x_byd=np.ascontiguousarray(xc.transpose(1, 2, 0)),  # (16,10,Nc)
            x_t0=np.ascontiguousarray(xc[:, 0, :].T),           # (10,Nc)
            **shared,
        ))
    return in_maps, pad, nact, Nc, h, C60, xp


def kernel(t, x, mask, W_embed, b_embed, W0, b0, W1, b1, W2, b2, W3, b3):
    global last_results
    from concourse import bass_utils

    dt_name = os.environ.get("KERNEL_DT", "f32r")
    mask = np.asarray(mask)
    B, Amax = mask.shape
    N = B * Amax

    in_maps, pad, nact, Nc, h, _, _ = _prep_host(
        t, x, mask, W_embed, b_embed, W0, b0, W1, b1, W2, b2, W3, b3, dt_name)

    res = None
    err = None
    for dve_dt in (True, False):
        try:
            nc = build_bass(Nc, dt_name, h, dve_writes_dt=dve_dt)
            res = bass_utils.run_bass_kernel_spmd(
                nc, in_maps, core_ids=list(range(N_CORES)))
            break
        except Exception as e:  # retry with conservative engine config
            err = e
            continue
    if res is None:
        raise err
    last_results = res

    zall = np.concatenate([r["zout"].T for r in res.results], 0)  # (total, E)
    out = np.zeros((N, E), np.float32)
    out[pad[:nact]] = zall[:nact]
    return out.reshape(B, Amax, E)
